# revision 1
# baseline (speedup 1.0000x reference)
"""MultiHeadAttention forward on 8 Trainium2 NeuronCores (Bass/Tile).

Problem (hardcoded): B=2, S=2048, D=1024, H=16, HD=64.
  qkv = x @ w_qkv.T + b_qkv ; per-head attention with softmax(q k^T/8 + mask);
  out = values @ w_out.T + b_out.

Sharding: tensor-parallel over heads -- core c owns heads {2c, 2c+1}
(value dims 128c..128c+127).  Each core computes its 2 heads end-to-end and
a partial output projection; the host sums the 8 partials and adds the
bias constant (b_out + b_v @ w_out.T, exact because softmax rows sum to 1).

Device layout notes:
 - scores are computed TRANSPOSED (S^T[k,tq] = K^T.T @ Q^T per head) so the
   softmax exp can run on ScalarE straight out of PSUM and feed the AV
   matmul without any transposes.
 - V is projected in bf16 directly into [token, feat] layout with an extra
   ones column; the AV matmul (lhsT = [v|1]) then produces both values^T
   and the softmax denominator l in one pass.
 - vext carries 32 ones columns, so the AV output rows 64..95 all hold the
   denominator l; a 32x32 DVE block transpose makes l partition-parallel for
   a cheap reciprocal, a second transpose brings 1/l back as a row, and a
   K=1 PE matmul broadcasts it across partitions for the DVE normalize.
   (No SBUF->SBUF shuffle DMAs: tiny partition-scatter HWDGE transfers were
   observed to wedge the NeuronCore for the next NEFF execution.)
 - matmuls use float32r (full PE rate for moving dim >= 256, ~1e-4 rel err).
 - q/k score matmuls for the two heads are emitted back-to-back on disjoint
   PE row groups (partitions 0-63 / 64-127) so they run concurrently.
"""
import sys
if "/opt/trn_rl_repo" not in sys.path:
    sys.path.insert(0, "/opt/trn_rl_repo")
import numpy as np

B, S, D, H = 2, 2048, 1024, 16
HD = D // H           # 64
NCORES = 8
T = B * S             # 4096 tokens
NB = S // 512         # 4 tq blocks per batch
NCH = S // 128        # 16 kpos chunks per batch

_CACHE = {}


def build_nc(use_mask: bool, reps: int = 1, debug_dump: bool = False):
    """Build + compile the per-core Bass program (SPMD-identical)."""
    import concourse.bacc as bacc
    import concourse.tile as tile
    from concourse import mybir

    f32 = mybir.dt.float32
    f32r = mybir.dt.float32r
    bf16 = mybir.dt.bfloat16
    EXP = mybir.ActivationFunctionType.Exp
    MULT = mybir.AluOpType.mult

    nc = bacc.Bacc("TRN2", target_bir_lowering=False, debug=False,
                   num_devices=NCORES)

    xT = nc.dram_tensor("xT", (D, T), f32r, kind="ExternalInput")
    xTb = nc.dram_tensor("xTb", (D, T), bf16, kind="ExternalInput")
    wqkT = nc.dram_tensor("wqkT", (D, 256), f32r, kind="ExternalInput")
    bqk = nc.dram_tensor("bqk", (128, 2), f32, kind="ExternalInput")
    wvT = nc.dram_tensor("wvT", (D, 128), bf16, kind="ExternalInput")
    woT = nc.dram_tensor("woT", (64, 2, D), f32r, kind="ExternalInput")
    onesd = nc.dram_tensor("onesd", (1, 65), f32r, kind="ExternalInput")
    if use_mask:
        maskT = nc.dram_tensor("maskT", (B, S, S), f32r, kind="ExternalInput")
        ident = nc.dram_tensor("ident", (128, 128), f32r, kind="ExternalInput")
    out = nc.dram_tensor("out", (T, D), f32, kind="ExternalOutput")
    import os
    _dbg = set(os.environ.get("DBG", "qkt,vals,rl").split(",")) if debug_dump else set()
    if "qkt" in _dbg:
        d_qkt = nc.dram_tensor("d_qkt", (128, 2, T), f32r, kind="ExternalOutput")
    if "vals" in _dbg:
        d_vals0 = nc.dram_tensor("d_vals0", (64, T), f32r, kind="ExternalOutput")
        d_vals1 = nc.dram_tensor("d_vals1", (64, T), f32r, kind="ExternalOutput")
    if "rl" in _dbg:
        d_rl = nc.dram_tensor("d_rl", (B * 2 * NB, 512), f32r, kind="ExternalOutput")

    with tile.TileContext(nc) as tc:
        with tc.tile_pool(name="sbp", bufs=1) as sbp, \
             tc.tile_pool(name="xtp", bufs=10) as xtp, \
             tc.tile_pool(name="xtbp", bufs=10) as xtbp, \
             tc.tile_pool(name="ptp", bufs=4) as ptp, \
             tc.tile_pool(name="lrp", bufs=2) as lrp, \
             tc.tile_pool(name="otp", bufs=4) as otp, \
             tc.tile_pool(name="mkp", bufs=4) as mkp, \
             tc.tile_pool(name="mmp", bufs=2, space="PSUM") as mmp, \
             tc.tile_pool(name="scp", bufs=2, space="PSUM") as scp, \
             tc.tile_pool(name="avp", bufs=2, space="PSUM") as avp:

            # --- persistent SBUF tensors ---
            qkt = sbp.tile([128, 2, T], f32r, name="qkt")        # [feat, {q,k}, tok]
            vext = sbp.tile([128, B, 2, NCH, HD + 32], bf16, name="vext")
            valsT0 = sbp.tile([64, T], f32r, name="valsT0")
            valsT1 = sbp.tile([64, T], f32r, name="valsT1")
            wqk_sb = sbp.tile([128, 8, 256], f32r, name="wqk_sb")
            wv_sb = sbp.tile([128, 8, 128], bf16, name="wv_sb")
            wo_sb = sbp.tile([64, 2, D], f32r, name="wo_sb")
            bqk_sb = sbp.tile([128, 2], f32, name="bqk_sb")
            ones_sb = sbp.tile([65, 65], f32r, name="ones_sb")
            if use_mask:
                id_sb = sbp.tile([128, 128], f32r, name="id_sb")
                nc.sync.dma_start(id_sb, ident[:, :])

            for c in range(8):
                nc.sync.dma_start(wqk_sb[:, c, :], wqkT[128 * c:128 * c + 128, :])
                nc.sync.dma_start(wv_sb[:, c, :], wvT[128 * c:128 * c + 128, :])
            nc.sync.dma_start(wo_sb, woT[:, :, :])
            nc.sync.dma_start(bqk_sb, bqk[:, :])
            for _op in range(65):
                nc.sync.dma_start(ones_sb[_op:_op + 1, :], onesd[:, :])
            nc.vector.memset(vext[:, :, :, :, HD:HD + 32], 1.0)

            for rep in range(reps):
                # ================= Phase A: projections =================
                for tb in range(8):          # 512-token blocks over all 4096
                    xts, xtbs = [], []
                    for c in range(8):       # D chunks
                        xt = xtp.tile([128, 512], f32r, tag="xt",
                                      name=f"xt_{rep}_{tb}_{c}")
                        nc.sync.dma_start(
                            xt, xT[128 * c:128 * c + 128, 512 * tb:512 * tb + 512])
                        xts.append(xt)
                        xtb_t = xtbp.tile([128, 512], bf16, tag="xtb",
                                          name=f"xtb_{rep}_{tb}_{c}")
                        nc.sync.dma_start(
                            xtb_t, xTb[128 * c:128 * c + 128, 512 * tb:512 * tb + 512])
                        xtbs.append(xtb_t)
                    # q/k projections: out [feat 128, tok 512]
                    for m in range(2):
                        acc = mmp.tile([128, 512], f32, tag="mm",
                                       name=f"qk_{rep}_{tb}_{m}")
                        for c in range(8):
                            nc.tensor.matmul(
                                acc, wqk_sb[:, c, 128 * m:128 * m + 128], xts[c],
                                start=(c == 0), stop=(c == 7))
                        nc.vector.tensor_scalar_add(
                            qkt[:, m, 512 * tb:512 * tb + 512], acc,
                            bqk_sb[:, m:m + 1])
                    # v projection: out [tok 128, vfeat 128] (bf16 inputs)
                    for u in range(4):
                        tt = 4 * tb + u
                        b, cc = tt // NCH, tt % NCH
                        vp = mmp.tile([128, 128], f32, tag="mm",
                                      name=f"vp_{rep}_{tt}")
                        for c in range(8):
                            nc.tensor.matmul(
                                vp, xtbs[c][:, 128 * u:128 * u + 128],
                                wv_sb[:, c, :], start=(c == 0), stop=(c == 7))
                        nc.vector.tensor_copy(vext[:, b, 0, cc, 0:HD], vp[:, 0:64])
                        nc.vector.tensor_copy(vext[:, b, 1, cc, 0:HD], vp[:, 64:128])

                # ============ Phase B: attention, + Phase C per batch ============
                for b in range(B):
                    for tqb in range(NB):
                        tq0 = S * b + 512 * tqb
                        q_aps = [qkt[64 * h:64 * h + 64, 0, tq0:tq0 + 512]
                                 for h in range(2)]
                        avs = [avp.tile([128, 512], f32, tag="av",
                                        name=f"av_{rep}_{b}_{h}_{tqb}")
                               for h in range(2)]
                        for c in range(NCH):
                            # one sc tile holds chunk c for BOTH heads; the two
                            # score matmuls hit disjoint PE row groups (d 0-63 /
                            # 64-127) and run concurrently.
                            sc = scp.tile([128, 1024], f32, tag="sc",
                                          name=f"sc_{rep}_{b}_{tqb}_{c}")
                            for h in range(2):
                                k_ap = qkt[64 * h:64 * h + 64, 1,
                                           S * b + 128 * c:S * b + 128 * c + 128]
                                nc.tensor.matmul(
                                    sc[:, 512 * h:512 * h + 512], k_ap, q_aps[h],
                                    start=True, stop=(not use_mask))
                            if use_mask:
                                mt = mkp.tile([128, 512], f32r, tag="mk",
                                              name=f"mk_{rep}_{b}_{tqb}_{c}")
                                nc.sync.dma_start(
                                    mt, maskT[b, 128 * c:128 * c + 128,
                                              512 * tqb:512 * tqb + 512])
                                for h in range(2):
                                    nc.tensor.matmul(
                                        sc[:, 512 * h:512 * h + 512], id_sb, mt,
                                        start=False, stop=True)
                            pt = ptp.tile([128, 1024], bf16, tag="pt",
                                          name=f"pt_{rep}_{b}_{tqb}_{c}")
                            nc.scalar.activation(pt, sc, EXP)
                            for h in range(2):
                                nc.tensor.matmul(
                                    avs[h][0:96, :], vext[:, b, h, c, :],
                                    pt[:, 512 * h:512 * h + 512],
                                    start=(c == 0), stop=(c == NCH - 1))
                        # --- normalize: values^T[:, tq] = av[0:64] / l ---
                        # av rows 64..95 all hold l (32 ones columns in vext);
                        # 32x32 DVE block transposes give a partition-parallel
                        # reciprocal without any DMA.
                        for h in range(2):
                            av = avs[h]
                            ls = lrp.tile([96, 512], f32, tag="ls",
                                          name=f"ls_{rep}_{b}_{h}_{tqb}")
                            nc.vector.tensor_copy(ls[64:96, :], av[64:96, :])
                            lt = lrp.tile([96, 512], f32, tag="lt",
                                          name=f"lt_{rep}_{b}_{h}_{tqb}")
                            nc.vector.transpose(lt[64:96, :], ls[64:96, :])
                            rlp = lrp.tile([96, 512], f32, tag="rlp",
                                           name=f"rlp_{rep}_{b}_{h}_{tqb}")
                            lt3 = lt[64:96, :].rearrange(
                                "p (a b) -> p a b", b=32)[:, :, 0:1]
                            rlp3 = rlp[64:96, :].rearrange(
                                "p (a b) -> p a b", b=32)[:, :, 0:1]
                            nc.vector.reciprocal(rlp3, lt3)
                            rlrowf = lrp.tile([96, 512], f32, tag="rlrowf",
                                              name=f"rlrowf_{rep}_{b}_{h}_{tqb}")
                            nc.vector.transpose(rlrowf[64:96, :], rlp[64:96, :])
                            rlrow = lrp.tile([65, 512], f32r, tag="rlrow",
                                             name=f"rlrow_{rep}_{b}_{h}_{tqb}")
                            nc.vector.tensor_copy(rlrow[64:65, :],
                                                  rlrowf[64:65, :])
                            if "rl" in _dbg and rep == 0:
                                u = (b * 2 + h) * NB + tqb
                                nc.sync.dma_start(d_rl[u:u + 1, :],
                                                  rlrow[64:65, :])
                            bc = mmp.tile([64, 512], f32, tag="mm",
                                          name=f"bc_{rep}_{b}_{h}_{tqb}")
                            nc.tensor.matmul(bc, ones_sb[64:65, 0:64],
                                             rlrow[64:65, :],
                                             start=True, stop=True)
                            bcs = lrp.tile([64, 512], f32, tag="bcs",
                                           name=f"bcs_{rep}_{b}_{h}_{tqb}")
                            nc.vector.tensor_copy(bcs, bc)
                            vt = valsT0 if h == 0 else valsT1
                            nc.vector.tensor_tensor(
                                vt[:, tq0:tq0 + 512], av[0:64, :], bcs, MULT)
                        # ---- Phase C interleaved: this tq-block's out rows ----
                        for nb in range(2):
                            for u in range(4):
                                t0 = tq0 + 128 * u
                                op = mmp.tile([128, 512], f32, tag="mm",
                                              name=f"op_{rep}_{b}_{tqb}_{nb}_{u}")
                                nc.tensor.matmul(
                                    op, valsT0[:, t0:t0 + 128],
                                    wo_sb[:, 0, 512 * nb:512 * nb + 512],
                                    start=True, stop=False)
                                nc.tensor.matmul(
                                    op, valsT1[:, t0:t0 + 128],
                                    wo_sb[:, 1, 512 * nb:512 * nb + 512],
                                    start=False, stop=True)
                                ot = otp.tile([128, 512], f32, tag="ot",
                                              name=f"ot_{rep}_{b}_{tqb}_{nb}_{u}")
                                nc.vector.tensor_copy(ot, op)
                                nc.sync.dma_start(
                                    out[t0:t0 + 128, 512 * nb:512 * nb + 512], ot)
            if "qkt" in _dbg:
                nc.sync.dma_start(d_qkt[:, :, :], qkt)
            if "vals" in _dbg:
                nc.sync.dma_start(d_vals0[:, :], valsT0)
                nc.sync.dma_start(d_vals1[:, :], valsT1)
    nc.compile()
    return nc


def make_in_maps(mha_x, self_mask, w_qkv, b_qkv, w_out, b_out, use_mask):
    """Host-side sharding / layout prep. Returns (in_maps, host_bias)."""
    import ml_dtypes
    bf = np.dtype(ml_dtypes.bfloat16)
    x = np.asarray(mha_x, np.float32).reshape(T, D)
    xT_np = np.ascontiguousarray(x.T)                   # [D, T]
    xTb_np = np.ascontiguousarray(xT_np.astype(bf))
    scale = 1.0 / np.sqrt(np.float32(HD))               # 1/8
    wqkv = np.asarray(w_qkv, np.float32)
    bqkv = np.asarray(b_qkv, np.float32)
    wout = np.asarray(w_out, np.float32)
    bout = np.asarray(b_out, np.float32)

    # reference packs w_qkv rows as [H, (q,k,v), HD]: head h's q rows are
    # wqkv[192h:192h+64], k rows +64, v rows +128.
    wq_rows = lambda h: wqkv[192 * h:192 * h + 64, :]
    wk_rows = lambda h: wqkv[192 * h + 64:192 * h + 128, :]
    wv_rows = lambda h: wqkv[192 * h + 128:192 * h + 192, :]
    bq_of = lambda h: bqkv[192 * h:192 * h + 64]
    bk_of = lambda h: bqkv[192 * h + 64:192 * h + 128]
    bv_of = lambda h: bqkv[192 * h + 128:192 * h + 192]

    in_maps = []
    for c in range(NCORES):
        h0, h1 = 2 * c, 2 * c + 1
        wq = np.concatenate([wq_rows(h0), wq_rows(h1)], 0) * scale
        wk = np.concatenate([wk_rows(h0), wk_rows(h1)], 0)
        wv = np.concatenate([wv_rows(h0), wv_rows(h1)], 0)
        m = {
            "xT": xT_np,
            "xTb": xTb_np,
            "wqkT": np.ascontiguousarray(np.concatenate([wq, wk], 0).T),
            "bqk": np.ascontiguousarray(
                np.stack([np.concatenate([bq_of(h0), bq_of(h1)]) * scale,
                          np.concatenate([bk_of(h0), bk_of(h1)])], 1)),
            "wvT": np.ascontiguousarray(wv.T.astype(bf)),
            "woT": np.ascontiguousarray(
                wout[:, 128 * c:128 * c + 128].T.reshape(2, 64, D).transpose(1, 0, 2)),
            "onesd": np.ones((1, 65), np.float32),
        }
        if use_mask:
            m["maskT"] = np.ascontiguousarray(
                np.asarray(self_mask, np.float32).transpose(0, 2, 1))
            m["ident"] = np.eye(128, dtype=np.float32)
        in_maps.append(m)

    b_v_full = np.concatenate([bv_of(h) for h in range(H)])
    host_bias = b_v_full @ wout.T + bout                # [D], exact
    return in_maps, host_bias


def kernel(**inputs):
    from concourse.bass_utils import run_bass_kernel_spmd
    self_mask = np.asarray(inputs["self_mask"], np.float32)
    use_mask = bool(np.any(self_mask))
    key = ("nc", use_mask)
    if key not in _CACHE:
        _CACHE[key] = build_nc(use_mask)
    nc = _CACHE[key]
    in_maps, host_bias = make_in_maps(
        inputs["mha_x"], self_mask, inputs["w_qkv"], inputs["b_qkv"],
        inputs["w_out"], inputs["b_out"], use_mask)
    res = run_bass_kernel_spmd(nc, in_maps, core_ids=list(range(NCORES)))
    acc = np.zeros((T, D), np.float32)
    for c in range(NCORES):
        acc += res.results[c]["out"]
    acc += host_bias[None, :]
    return acc.reshape(B, S, D)



# revision 8
# speedup vs baseline: 6.2004x; 6.2004x over previous
"""MultiHeadAttention forward on 8 Trainium2 NeuronCores (Bass/Tile), v2.

Problem (hardcoded): B=2, S=2048, D=1024, H=16, HD=64.
  qkv = x @ w_qkv.T + b_qkv ; per-head attention with softmax(q k^T/8 + mask);
  out = values @ w_out.T + b_out.

Sharding: tensor-parallel over heads -- core c owns heads {2c, 2c+1}
(value dims 128c..128c+127).  Each core computes its 2 heads end-to-end and
a partial output projection in bf16; the host sums the 8 partials and adds
the bias constant (b_out + b_v @ w_out.T, exact because softmax rows sum
to 1).

v2 design notes (vs the v1 baseline this evolved from):
 - Everything is bf16 on the wire and in SBUF (x, q/k, v, probs, vals, w):
   halves DMA traffic and DVE element counts; matmuls run 1 cyc/row with
   FWL weight loads.  PSUM accumulation stays f32.
 - x is loaded once (16 x 512KB DMAs, batch-0 chunks first) and stays
   SBUF-resident; the ones row/columns come from memsets, not DMAs.
   Weight DMAs go out on the scalar-engine HWDGE queue in 4 batched
   transfers so the sync queue starts on x immediately (v1 spent ~70us
   serially issuing 65 tiny DMAs before any compute).
 - Scores are computed transposed (S^T = K^T.T @ Q^T per head) so exp runs
   on ScalarE straight out of PSUM; the two heads' score matmuls sit on
   disjoint PE row groups (partitions 0-63 / 64-127) and run concurrently.
 - V carries ones columns ([64v|32ones] for head 0, [32ones|64v] for head
   1, written at PSUM base 0 / 32) so one AV matmul per head yields both
   values^T and the softmax denominator l; head 1's values land on PSUM
   partitions 64:128, so the two heads' normalized values form a single
   [128, tq] vals tile and the output projection is one K=128 matmul per
   128-token block (v1 needed two K=64 matmuls).
 - l sits on a single partition row; a direct DVE reciprocal on the [1,512]
   row replaces v1's 32x32 transpose dance, and a K=1 matmul against a
   memset ones row broadcasts 1/l across partitions for the normalize
   multiply.
 - The emission order software-pipelines the whole kernel: Phase A (b=0)
   is interleaved with the first attention block's chunk loop, and a
   deferred-thunk queue drips Phase A (b=1) and the previous block's
   output projection into later chunk loops, so the PE never idles long
   enough for the HAM clock gate to re-throttle it to 1.2 GHz (v1 ran
   ~59% of its span at half clock) while ScalarE exp (the Phase B floor,
   ~1.15us per 128x1024 chunk) stays saturated.
"""
import sys
if "/opt/trn_rl_repo" not in sys.path:
    sys.path.insert(0, "/opt/trn_rl_repo")
import numpy as np

B, S, D, H = 2, 2048, 1024, 16
HD = D // H           # 64
NCORES = 8
T = B * S             # 4096 tokens
NB = S // 512         # 4 tq blocks per batch
NCH = S // 128        # 16 kpos chunks per batch

_CACHE = {}


def build_nc(use_mask: bool, reps: int = 1):
    """Build + compile the per-core Bass program (SPMD-identical)."""
    import concourse.bacc as bacc
    import concourse.tile as tile
    from concourse import mybir

    f32 = mybir.dt.float32
    f32r = mybir.dt.float32r
    bf16 = mybir.dt.bfloat16
    EXP = mybir.ActivationFunctionType.Exp
    MULT = mybir.AluOpType.mult

    nc = bacc.Bacc("TRN2", target_bir_lowering=False, debug=False,
                   num_devices=NCORES)

    xTb = nc.dram_tensor("xTb", (128, 8, B, S), bf16, kind="ExternalInput")
    wqk = nc.dram_tensor("wqk", (128, 8, 256), bf16, kind="ExternalInput")
    wv = nc.dram_tensor("wv", (128, 8, 128), bf16, kind="ExternalInput")
    wo = nc.dram_tensor("wo", (128, D), bf16, kind="ExternalInput")
    bqk = nc.dram_tensor("bqk", (128, 2), f32, kind="ExternalInput")
    if use_mask:
        maskT = nc.dram_tensor("maskT", (B, S, S), f32r, kind="ExternalInput")
        ident = nc.dram_tensor("ident", (128, 128), f32r, kind="ExternalInput")
    out = nc.dram_tensor("out", (T, D), bf16, kind="ExternalOutput")

    with tile.TileContext(nc) as tc:
        with tc.tile_pool(name="sbp", bufs=1) as sbp, \
             tc.tile_pool(name="ptp", bufs=4) as ptp, \
             tc.tile_pool(name="otp", bufs=2) as otp, \
             tc.tile_pool(name="rlp", bufs=2) as rlp, \
             tc.tile_pool(name="bcp", bufs=2) as bcp, \
             tc.tile_pool(name="mkp", bufs=4) as mkp, \
             tc.tile_pool(name="mmp", bufs=2, space="PSUM") as mmp, \
             tc.tile_pool(name="scp", bufs=2, space="PSUM") as scp, \
             tc.tile_pool(name="avp", bufs=2, space="PSUM") as avp:

            # --- persistent SBUF tensors (separate tiles per block so the
            # Tile dependency tracker never sees false cross-block deps) ---
            xb = [[sbp.tile([128, S], bf16, name=f"xb_{cc}_{b}")
                   for b in range(B)] for cc in range(8)]
            qkt = [[sbp.tile([128, 2, 512], bf16, name=f"qkt_{b}_{tb}")
                    for tb in range(4)] for b in range(B)]
            vext = [[sbp.tile([128, 2, 4, 128], bf16, name=f"vext_{b}_{tb}")
                     for tb in range(4)] for b in range(B)]
            vals = [[sbp.tile([128, 512], bf16, name=f"vals_{b}_{tqb}")
                     for tqb in range(NB)] for b in range(B)]
            wqk_sb = sbp.tile([128, 8, 256], bf16, name="wqk_sb")
            wv_sb = sbp.tile([128, 8, 128], bf16, name="wv_sb")
            wo_sb = sbp.tile([128, D], bf16, name="wo_sb")
            bqk_sb = sbp.tile([128, 2], f32, name="bqk_sb")
            ones_sb = sbp.tile([65, 64], bf16, name="ones_sb")
            if use_mask:
                id_sb = sbp.tile([128, 128], f32r, name="id_sb")

            # weight loads on the scalar-engine HWDGE queue (parallel with
            # the x loads below on the sync queue)
            nc.scalar.dma_start(wqk_sb, wqk[:, :, :])
            nc.scalar.dma_start(wv_sb, wv[:, :, :])
            nc.scalar.dma_start(wo_sb, wo[:, :])
            nc.scalar.dma_start(bqk_sb, bqk[:, :])
            if use_mask:
                nc.scalar.dma_start(id_sb, ident[:, :])

            nc.vector.memset(ones_sb, 1.0)
            for b in range(B):
                for tb in range(4):
                    nc.vector.memset(vext[b][tb][:, 0, :, 64:128], 1.0)
                    nc.vector.memset(vext[b][tb][:, 1, :, 0:64], 1.0)

            for rep in range(reps):
                # x loads: batch 0's feature chunks first so Phase A can
                # start as soon as chunk (0,0) lands
                if rep == 0:
                    for b in range(B):
                        for cc in range(8):
                            nc.sync.dma_start(xb[cc][b], xTb[:, cc, b, :])

                deferred = []

                def pop_def(n):
                    for _ in range(min(n, len(deferred))):
                        deferred.pop(0)()

                def emit_tb(b, tb, defer):
                    """Phase A for 512-token block tb of batch b.
                    defer=False emits now; True appends small thunks."""
                    t0 = 512 * tb
                    thunks = []

                    def qk_group(m):
                        cell = {}

                        def start():
                            cell["acc"] = mmp.tile(
                                [128, 512], f32, tag="mm",
                                name=f"qk_{rep}_{b}_{tb}_{m}")

                        def mm(cc):
                            nc.tensor.matmul(
                                cell["acc"],
                                wqk_sb[:, cc, 128 * m:128 * m + 128],
                                xb[cc][b][:, t0:t0 + 512],
                                start=(cc == 0), stop=(cc == 7))

                        def fin():
                            nc.vector.tensor_scalar_add(
                                qkt[b][tb][:, m, :], cell["acc"],
                                bqk_sb[:, m:m + 1])
                        return start, mm, fin

                    def v_group(u):
                        cell = {}

                        def start():
                            cell["vp"] = mmp.tile(
                                [128, 512], f32, tag="mm",
                                name=f"vp_{rep}_{b}_{tb}_{u}")

                        def mm(cc):
                            nc.tensor.matmul(
                                cell["vp"][:, 0:128],
                                xb[cc][b][:, t0 + 128 * u:t0 + 128 * u + 128],
                                wv_sb[:, cc, :],
                                start=(cc == 0), stop=(cc == 7))

                        def fin():
                            nc.vector.tensor_copy(
                                vext[b][tb][:, 0, u, 0:64], cell["vp"][:, 0:64])
                            nc.vector.tensor_copy(
                                vext[b][tb][:, 1, u, 64:128], cell["vp"][:, 64:128])
                        return start, mm, fin

                    groups = [qk_group(0), qk_group(1)] + \
                             [v_group(u) for u in range(4)]
                    for (start, mm, fin) in groups:
                        # thunks of 2 matmuls keep PE backlog jitter small
                        for cc0 in range(0, 8, 2):
                            def th(start=start, mm=mm, cc0=cc0):
                                if cc0 == 0:
                                    start()
                                mm(cc0)
                                mm(cc0 + 1)
                            thunks.append(th)
                        thunks.append(fin)
                    if defer:
                        deferred.extend(thunks)
                    else:
                        for th in thunks:
                            th()

                def emit_chunk(b, tqb, c, q_aps, avs):
                    sc = scp.tile([128, 1024], f32, tag="sc",
                                  name=f"sc_{rep}_{b}_{tqb}_{c}")
                    for h in range(2):
                        k_ap = qkt[b][c // 4][64 * h:64 * h + 64, 1,
                                             128 * (c % 4):128 * (c % 4) + 128]
                        nc.tensor.matmul(sc[:, 512 * h:512 * h + 512],
                                         k_ap, q_aps[h],
                                         start=True, stop=(not use_mask))
                    if use_mask:
                        mt = mkp.tile([128, 512], f32r, tag="mk",
                                      name=f"mk_{rep}_{b}_{tqb}_{c}")
                        nc.sync.dma_start(
                            mt, maskT[b, 128 * c:128 * c + 128,
                                      512 * tqb:512 * tqb + 512])
                        for h in range(2):
                            nc.tensor.matmul(sc[:, 512 * h:512 * h + 512],
                                             id_sb, mt, start=False, stop=True)
                    pt = ptp.tile([128, 1024], bf16, tag="pt",
                                  name=f"pt_{rep}_{b}_{tqb}_{c}")
                    nc.scalar.activation(pt, sc, EXP)
                    for h in range(2):
                        nc.tensor.matmul(avs[h][:, :],
                                         vext[b][c // 4][:, h, c % 4, :],
                                         pt[:, 512 * h:512 * h + 512],
                                         start=(c == 0), stop=(c == NCH - 1))

                def emit_normalize(b, tqb, avs):
                    for h in range(2):
                        av = avs[h]
                        lrow = 64 if h == 0 else 32   # the denominator row
                        vlo = 0 if h == 0 else 64     # values partition base
                        rl = rlp.tile([65, 512], bf16, tag="rl",
                                      name=f"rl_{rep}_{b}_{tqb}_{h}")
                        with nc.allow_low_precision("softmax denom in bf16"):
                            nc.vector.reciprocal(rl[lrow:lrow + 1, :],
                                                 av[lrow:lrow + 1, :])
                        bct = mmp.tile([128, 512], f32, tag="mm",
                                       name=f"bc_{rep}_{b}_{tqb}_{h}")
                        nc.tensor.matmul(bct[vlo:vlo + 64, :],
                                         ones_sb[lrow:lrow + 1, 0:64],
                                         rl[lrow:lrow + 1, :],
                                         start=True, stop=True)
                        bcs = bcp.tile([128, 512], f32, tag="bcs",
                                       name=f"bcs_{rep}_{b}_{tqb}_{h}")
                        nc.vector.tensor_copy(bcs[vlo:vlo + 64, :],
                                              bct[vlo:vlo + 64, :])
                        nc.vector.tensor_tensor(
                            vals[b][tqb][vlo:vlo + 64, :],
                            av[vlo:vlo + 64, :], bcs[vlo:vlo + 64, :], MULT)

                def defer_phase_c(b, tqb):
                    t0g = S * b + 512 * tqb
                    cell = {}

                    def oalloc():
                        cell["ot"] = otp.tile([128, 4, D], bf16, tag="ot",
                                              name=f"ot_{rep}_{b}_{tqb}")
                    for u in range(4):
                        for nb2 in range(2):
                            def th(u=u, nb2=nb2):
                                if u == 0 and nb2 == 0:
                                    oalloc()
                                op = mmp.tile([128, 512], f32, tag="mm",
                                              name=f"op_{rep}_{b}_{tqb}_{u}_{nb2}")
                                nc.tensor.matmul(
                                    op, vals[b][tqb][:, 128 * u:128 * u + 128],
                                    wo_sb[:, 512 * nb2:512 * nb2 + 512],
                                    start=True, stop=True)
                                nc.vector.tensor_copy(
                                    cell["ot"][:, u, 512 * nb2:512 * nb2 + 512],
                                    op)
                            deferred.append(th)

                    def dth():
                        nc.sync.dma_start(
                            out[t0g:t0g + 512, :].rearrange(
                                "(u p) f -> p u f", p=128), cell["ot"])
                    deferred.append(dth)

                def emit_tqb(b, tqb, head_interleave=False):
                    if b == 1 and tqb == 0:
                        # batch 1's attention reads qkt[1]/vext[1] produced by
                        # the deferred Phase A(b=1) thunks -- every one of
                        # them must be EMITTED (program order = dependency
                        # order for Tile) before these chunks are.
                        pop_def(len(deferred))
                    q_aps = [qkt[b][tqb][64 * h:64 * h + 64, 0, :]
                             for h in range(2)]
                    avs = [avp.tile([128, 512], f32, tag="av",
                                    name=f"av_{rep}_{b}_{tqb}_{h}")
                           for h in range(2)]
                    for c in range(NCH):
                        if head_interleave and c % 4 == 0 and c > 0:
                            emit_tb(0, c // 4, defer=False)
                        emit_chunk(b, tqb, c, q_aps, avs)
                        if not head_interleave:
                            pop_def(4)
                    emit_normalize(b, tqb, avs)
                    defer_phase_c(b, tqb)

                # ---- head: Phase A (b=0) interleaved with (0, tqb0) ----
                emit_tb(0, 0, defer=False)
                emit_tqb(0, 0, head_interleave=True)
                # ---- steady state ----
                for b in range(B):
                    for tqb in range(NB):
                        if b == 0 and tqb == 0:
                            continue
                        if b == 0 and tqb == 1:
                            for tb in range(4):
                                emit_tb(1, tb, defer=True)
                        emit_tqb(b, tqb)
                while deferred:
                    pop_def(8)
    nc.compile()
    return nc


def make_in_maps(mha_x, self_mask, w_qkv, b_qkv, w_out, b_out, use_mask):
    """Host-side sharding / layout prep. Returns (in_maps, host_bias)."""
    import ml_dtypes
    bf = np.dtype(ml_dtypes.bfloat16)
    x = np.asarray(mha_x, np.float32).reshape(T, D)
    xT_np = np.ascontiguousarray(x.T)                   # [D, T]
    # [128, 8(cc), B, S] with xTb[p, cc, b, t] = x[2048b + t, 128cc + p]
    xTb_np = np.ascontiguousarray(
        xT_np.reshape(8, 128, B, S).transpose(1, 0, 2, 3).astype(bf))
    scale = 1.0 / np.sqrt(np.float32(HD))               # 1/8
    wqkv = np.asarray(w_qkv, np.float32)
    bqkv = np.asarray(b_qkv, np.float32)
    wout = np.asarray(w_out, np.float32)
    bout = np.asarray(b_out, np.float32)

    # reference packs w_qkv rows as [H, (q,k,v), HD]: head h's q rows are
    # wqkv[192h:192h+64], k rows +64, v rows +128.
    wq_rows = lambda h: wqkv[192 * h:192 * h + 64, :]
    wk_rows = lambda h: wqkv[192 * h + 64:192 * h + 128, :]
    wv_rows = lambda h: wqkv[192 * h + 128:192 * h + 192, :]
    bq_of = lambda h: bqkv[192 * h:192 * h + 64]
    bk_of = lambda h: bqkv[192 * h + 64:192 * h + 128]
    bv_of = lambda h: bqkv[192 * h + 128:192 * h + 192]

    in_maps = []
    for c in range(NCORES):
        h0, h1 = 2 * c, 2 * c + 1
        wq = np.concatenate([wq_rows(h0), wq_rows(h1)], 0) * scale
        wk = np.concatenate([wk_rows(h0), wk_rows(h1)], 0)
        wvm = np.concatenate([wv_rows(h0), wv_rows(h1)], 0)
        wqkT = np.concatenate([wq, wk], 0).T            # [1024, 256]
        wvT = wvm.T                                     # [1024, 128]
        m = {
            "xTb": xTb_np,
            "wqk": np.ascontiguousarray(
                wqkT.reshape(8, 128, 256).transpose(1, 0, 2).astype(bf)),
            "wv": np.ascontiguousarray(
                wvT.reshape(8, 128, 128).transpose(1, 0, 2).astype(bf)),
            "wo": np.ascontiguousarray(
                wout[:, 128 * c:128 * c + 128].T.astype(bf)),
            "bqk": np.ascontiguousarray(
                np.stack([np.concatenate([bq_of(h0), bq_of(h1)]) * scale,
                          np.concatenate([bk_of(h0), bk_of(h1)])], 1)),
        }
        if use_mask:
            m["maskT"] = np.ascontiguousarray(
                np.asarray(self_mask, np.float32).transpose(0, 2, 1))
            m["ident"] = np.eye(128, dtype=np.float32)
        in_maps.append(m)

    b_v_full = np.concatenate([bv_of(h) for h in range(H)])
    host_bias = b_v_full @ wout.T + bout                # [D], exact
    return in_maps, host_bias


def kernel(**inputs):
    from concourse.bass_utils import run_bass_kernel_spmd
    self_mask = np.asarray(inputs["self_mask"], np.float32)
    use_mask = bool(np.any(self_mask))
    key = ("nc", use_mask)
    if key not in _CACHE:
        _CACHE[key] = build_nc(use_mask)
    nc = _CACHE[key]
    in_maps, host_bias = make_in_maps(
        inputs["mha_x"], self_mask, inputs["w_qkv"], inputs["b_qkv"],
        inputs["w_out"], inputs["b_out"], use_mask)
    res = run_bass_kernel_spmd(nc, in_maps, core_ids=list(range(NCORES)))
    acc = np.zeros((T, D), np.float32)
    for c in range(NCORES):
        acc += np.asarray(res.results[c]["out"], np.float32)
    acc += host_bias[None, :]
    return acc.reshape(B, S, D)
